# revision 52
# baseline (speedup 1.0000x reference)
"""Trainium2 Bass kernel for LocalSelfAttention (sliding-window attention).

Reference computation (fp32):
  qkv = x @ W_qkv ; q /= 8 ; sliding window of 7 keys (3 each side, zero-padded)
  attn = softmax(q . k_win + pos_bias) ; out = (attn @ v_win) @ W_out

Sharding: data-parallel over B*HW = 128 independent rows -> 16 rows per core.
Each core processes its rows in 8 pairs (512 tokens per pair).

Per-core layout, software-pipelined 5 pairs deep so the PE never stalls on
the vector-engine softmax chain:
  stage A1(p): xT arrives via DMA-transpose (x pre-cast to bf16 on host);
               qkT = W_qk^T. @ xT in fp8 DoubleRow (2 rows/cycle);
               V = xT^T. @ W_v (bf16)
  stage A2(p): scores ST[key,q] per head; exp w/ folded 1/sqrt(dk) and fp8
               descale (Scalar); * expB band mask (Pool)
  stage B1(p): denom = ones^T. @ attn_un (replicated across 64 partitions,
               2 heads per psum tile) + rank-2 matmul U^T. @ E adding the
               zero-pad edge correction; reciprocal_approx_fast from PSUM
  stage B2(p): avT = V^T. @ attn_un; * recip while copying PSUM->SBUF
  stage C(p):  out = avT^T. @ W_out -> DMA

Steady-state emission per step t: A1(t), then scores(t-1) interleaved
head-by-head with den(t-2)/av(t-3) groups (covers PSUM-rotation waits with
PE work), then out-proj C(t-4).
"""

import numpy as np
import ml_dtypes

import concourse.bass as bass
import concourse.tile as tile
from concourse import bacc, mybir
from concourse.bass_utils import run_bass_kernel_spmd

# Problem constants (hardcoded per contract)
B, HW, S, D = 2, 64, 256, 512
HEADS, DK, KSIZE, PAD = 8, 64, 7, 3
HDK = HEADS * DK            # 512
QK = 2 * HDK                # 1024 (q and k dims)
N_CORES = 8
ROWS_PER_CORE = (B * HW) // N_CORES   # 16
PAIRS = ROWS_PER_CORE // 2            # 8
PTOK = 2 * S                          # 512 tokens per pair
P = 128
NCH = S // P                          # 2 key chunks per row
STRIPE = 132                          # query stripe width per key chunk (even)
STRIPE_PAD = 256                      # psum slot per (chunk,row) stripe, bank aligned
STRIPE_START = (0, S - STRIPE)        # stripe start per chunk within a row
HPAIRS = HEADS // 2                   # 4 head pairs packed on 64+64 partitions

F32 = mybir.dt.float32
BF16 = mybir.dt.bfloat16
FP8 = mybir.dt.float8e4
FP8_WSCALE = 32.0                     # fp8 qk weight pre-scale (power of 2)
EXP_SCALE = 1.0 / (FP8_WSCALE * FP8_WSCALE * 8.0)  # undo w-scales + 1/sqrt(DK)

_CACHE = {}


def _host_constants(pos_bias, W_qkv, W_out):
    """Host-precomputed tensors: fp8 qk weights, bf16 v/out weights, expB
    band mask, rank-2 edge-correction factors."""
    W1 = W_qkv.astype(np.float32).copy()
    # qk weights scaled up by 32 to sit in fp8e4m3's normal range (std 0.02
    # would straddle the 2^-6 min normal); compensated in the exp scale,
    # which also folds in the 1/sqrt(DK) query scaling.
    W1qk = (W1[:, :QK] * FP8_WSCALE).astype(ml_dtypes.float8_e4m3)  # [512, 1024]
    W1v = W1[:, QK:].astype(ml_dtypes.bfloat16)               # [512, 512]
    W2 = W_out.astype(np.float32).astype(ml_dtypes.bfloat16)  # [512, 512]

    pb = pos_bias.astype(np.float32)              # [H, S, KSIZE]
    # expB[j, h, c, q'] : key j (within chunk c), query q = STRIPE_START[c] + q'
    # value exp(pos_bias[h, q, w]) with w = (j_global - q) + PAD if in band else 0
    j = np.arange(P)[:, None, None, None]
    h = np.arange(HEADS)[None, :, None, None]
    c = np.arange(NCH)[None, None, :, None]
    qp = np.arange(STRIPE)[None, None, None, :]
    q_glob = np.array(STRIPE_START)[None, None, :, None] + qp
    j_glob = c * P + j
    w = j_glob - q_glob + PAD
    in_band = (w >= 0) & (w < KSIZE)
    w_c = np.clip(w, 0, KSIZE - 1)
    bias_val = pb[h, q_glob, w_c]
    expB = np.where(in_band, np.exp(bias_val), 0.0).astype(np.float32)
    expB = expB.astype(ml_dtypes.bfloat16)        # [128, H, NCH, STRIPE]

    # edge correction: sum over out-of-range window slots of exp(bias).
    # Shipped as the moving operand E of a rank-2 matmul U^T. @ E that
    # accumulates it straight into the denominator PSUM: row i of E is the
    # correction for head 2j+i, row i of U selects partition half i.
    q = np.arange(S)[None, :, None]
    w2 = np.arange(KSIZE)[None, None, :]
    oor = ((q + w2 - PAD) < 0) | ((q + w2 - PAD) >= S)
    ec = (np.exp(pb) * oor).sum(-1)               # [H, S]
    ec_pair = np.concatenate([ec, ec], axis=1)    # [H, PTOK]
    ecE = np.empty((2, HPAIRS, PTOK), np.float32)
    for jj in range(HPAIRS):
        ecE[0, jj, :] = ec_pair[2 * jj]
        ecE[1, jj, :] = ec_pair[2 * jj + 1]
    return W1qk, W1v, W2, expB, ecE.astype(ml_dtypes.bfloat16)


def _build_nc():
    nc = bacc.Bacc(None, target_bir_lowering=False)
    x_d = nc.dram_tensor("x", [ROWS_PER_CORE * S, D], BF16, kind="ExternalInput")
    w1qk_d = nc.dram_tensor("w1qk", [D, QK], FP8, kind="ExternalInput")
    w1v_d = nc.dram_tensor("w1v", [D, HDK], BF16, kind="ExternalInput")
    w2_d = nc.dram_tensor("w2", [HDK, D], BF16, kind="ExternalInput")
    expb_d = nc.dram_tensor("expb", [P, HEADS, NCH, STRIPE], BF16, kind="ExternalInput")
    ec_d = nc.dram_tensor("ec", [2, HPAIRS, PTOK], BF16, kind="ExternalInput")
    u_d = nc.dram_tensor("u", [2, P], BF16, kind="ExternalInput")
    out_d = nc.dram_tensor("out", [ROWS_PER_CORE * S, D], BF16, kind="ExternalOutput")

    KO = D // P      # 4 K-chunks for projections
    TC = PTOK // P   # 4 token chunks per pair
    QKC = QK // P    # 8 qk output chunks
    HC = HDK // P    # 4 hdk chunks
    DR = mybir.MatmulPerfMode.DoubleRow
    COPY = mybir.ActivationFunctionType.Copy

    with tile.TileContext(nc) as tc:
        with (
            tc.tile_pool(name="const", bufs=1) as const,
            tc.tile_pool(name="io", bufs=3) as io,
            tc.tile_pool(name="early", bufs=2) as early,
            tc.tile_pool(name="vpool", bufs=3) as vpool,
            tc.tile_pool(name="attn", bufs=3) as attnp,
            tc.tile_pool(name="bpool", bufs=2) as bpool,
            tc.tile_pool(name="ps_proj", bufs=3, space="PSUM") as ps_proj,
            tc.tile_pool(name="ps_st", bufs=2, space="PSUM") as ps_st,
        ):
            # ---- constants; first x transpose goes ahead of the fat consts
            # (everything rides the sync queue, scalar stays free for copies)
            w1qk_sb = const.tile([P, KO, QK], FP8)
            w1v_sb = const.tile([P, KO, HDK], BF16)
            expb_sb = const.tile([P, HEADS, NCH, STRIPE], BF16)
            w2_sb = const.tile([P, HC, D], BF16)
            ecE_sb = const.tile([2, HPAIRS, PTOK], BF16)
            u_sb = const.tile([2, P], BF16)
            ones_sb = const.tile([P, 64], BF16)
            nc.vector.memset(ones_sb, 1.0)

            def load_w1qk():
                # scalar hwdge queue: weight transfers run in parallel with
                # the x transposes on the sync queue (per-queue in-order)
                nc.scalar.dma_start(
                    w1qk_sb[:], w1qk_d.rearrange("(ko ki) n -> ki ko n", ki=P))

            def load_w1v():
                nc.scalar.dma_start(
                    w1v_sb[:], w1v_d.rearrange("(ko ki) n -> ki ko n", ki=P))

            warm = {}

            def warmup_pe():
                # dummy matmuls during the initial DMA wait: the PE needs ~3us
                # of continuous busy to reach its full 2.4GHz p-state
                scratch = const.tile([P, PTOK], BF16, name="warm_scr")
                nc.gpsimd.memset(scratch, 0.0)
                wps = ps_proj.tile([P, PTOK], F32, tag="warm", bufs=1, name="wps")
                warm["scratch"], warm["wps"] = scratch, wps
                for i in range(11):
                    nc.tensor.matmul(
                        wps[0:64, :], ones_sb[:], scratch[:],
                        start=True, stop=True,
                    )

            def pe_filler(n):
                # keep the PE busy (and its p-state hot) through pipeline-fill
                # bubbles where no other matmul work exists yet
                for i in range(n):
                    nc.tensor.matmul(
                        warm["wps"][0:64, :], ones_sb[:], warm["scratch"][:],
                        start=True, stop=True,
                    )

            def load_consts_rest():
                nc.sync.dma_start(expb_sb[:], expb_d[:])
                nc.sync.dma_start(
                    w2_sb[:], w2_d.rearrange("(hc ki) n -> ki hc n", ki=P))
                nc.sync.dma_start(ecE_sb[:], ec_d[:])
                nc.sync.dma_start(u_sb[:], u_d[:])

            xT_tiles = {}
            qkT_tiles = {}
            attn_tiles = {}
            recip_tiles = {}
            v_tiles = {}
            avT_tiles = {}

            def stage_load(pr, interleave=None):
                # DMA-transpose: x [tokens, D] bf16 -> xT[p, ko, t] = x[t, ko*128+p].
                # Two half-transposes so the fp8 cast can chase the transfer;
                # `interleave` slots a const DMA between them (in-order queue).
                xT = io.tile([P, KO, PTOK], BF16, tag="xT")
                for hf in range(2):
                    nc.sync.dma_start_transpose(
                        xT[:, :, hf * S:(hf + 1) * S],
                        x_d[pr * PTOK + hf * S:pr * PTOK + (hf + 1) * S, :])
                    if interleave:
                        interleave[hf]()
                xT_tiles[pr] = xT

            def stage_a1(pr):
                xT = xT_tiles.pop(pr)
                xT8 = early.tile([P, KO, PTOK], FP8, tag="xT8")
                for hf in range(2):
                    nc.vector.tensor_copy(
                        xT8[:, :, hf * S:(hf + 1) * S],
                        xT[:, :, hf * S:(hf + 1) * S])

                # qk projection in fp8 DoubleRow: qkT [qk dims, tokens]
                qkT = early.tile([P, QKC, PTOK], BF16, tag="qkT", bufs=3)
                qkT_tiles[pr] = qkT
                for m in range(QKC):
                    pp = ps_proj.tile([P, PTOK], F32, tag="p512")
                    for kp in range(KO // 2):
                        nc.tensor.matmul(
                            pp[:],
                            w1qk_sb[:, 2 * kp:2 * kp + 2, m * P:(m + 1) * P],
                            xT8[:, 2 * kp:2 * kp + 2, :],
                            start=(kp == 0), stop=(kp == KO // 2 - 1),
                            perf_mode=DR,
                        )
                    if m % 2 == 0:
                        nc.scalar.activation(qkT[:, m, :], pp[:], func=COPY)
                    else:
                        nc.vector.tensor_copy(qkT[:, m, :], pp[:])

                # v projection (bf16): V [tokens, hdk]
                v_sb = vpool.tile([P, TC, HDK], BF16, tag="v_sb", bufs=4)
                v_tiles[pr] = v_sb
                for tcc in range(TC):
                    pp = ps_proj.tile([P, PTOK], F32, tag="p512")
                    for ko in range(KO):
                        nc.tensor.matmul(
                            pp[:],
                            xT[:, ko, tcc * P:(tcc + 1) * P],
                            w1v_sb[:, ko, :],
                            start=(ko == 0), stop=(ko == KO - 1),
                        )
                    if tcc % 2 == 0:
                        nc.scalar.activation(v_sb[:, tcc, :], pp[:], func=COPY)
                    else:
                        nc.vector.tensor_copy(v_sb[:, tcc, :], pp[:])

            def scores_head(pr, h):
                # one head's scores + exp + band mask
                qkT = qkT_tiles[pr]
                mq = h // 2          # q chunk index in qkT
                mk = 4 + h // 2      # k chunk index in qkT
                p0 = 64 * (h % 2)    # partition offset within chunk
                sl = slice(p0, p0 + 64)

                st = ps_st.tile([P, NCH, 2, STRIPE_PAD], F32, tag="st")
                for c in range(NCH):
                    for r in range(2):
                        nc.tensor.matmul(
                            st[:, c, r, :STRIPE],
                            qkT[sl, mk, r * S + c * P:r * S + (c + 1) * P],
                            qkT[sl, mq,
                                r * S + STRIPE_START[c]:
                                r * S + STRIPE_START[c] + STRIPE],
                            start=True, stop=True,
                        )
                attn_un = attnp.tile([P, NCH, 2, STRIPE], BF16, tag=f"attn_un{h}")
                nc.scalar.activation(
                    attn_un[:], st[:, :, :, :STRIPE],
                    func=mybir.ActivationFunctionType.Exp,
                    scale=EXP_SCALE)
                nc.gpsimd.tensor_tensor(
                    attn_un[:], attn_un[:],
                    expb_sb[:, h, :, None, :].to_broadcast((P, NCH, 2, STRIPE)),
                    mybir.AluOpType.mult,
                )
                attn_tiles.setdefault(pr, []).append(attn_un)

            def den_group(pr, j):
                # denominators for head pair j: ones-matmuls + rank-2 edge
                # correction accumulated in PSUM, then approx reciprocal
                attn_uns = attn_tiles[pr]
                if j == 0:
                    recip_tiles[pr] = bpool.tile(
                        [P, HPAIRS, PTOK], F32, tag="recip_rep",
                        name="recip_rep")
                recip_rep = recip_tiles[pr]
                den = ps_proj.tile([P, PTOK], F32, tag="p512")
                for h in (2 * j, 2 * j + 1):
                    p0 = 64 * (h % 2)
                    sl = slice(p0, p0 + 64)
                    tpos = None if p0 == 0 else (0, 64)
                    for r in range(2):
                        for c in range(NCH):
                            nc.tensor.matmul(
                                den[sl, r * S + STRIPE_START[c]:
                                        r * S + STRIPE_START[c] + STRIPE],
                                ones_sb[:],
                                attn_uns[h][:, c, r, :],
                                start=(r == 0 and c == 0),
                                stop=False,
                                tile_position=tpos,
                            )
                nc.tensor.matmul(
                    den[:], u_sb[:, :], ecE_sb[:, j, :],
                    start=False, stop=True,
                )
                nc.vector.reciprocal_approx_fast(recip_rep[:, j, :], den[:])

            def av_group(pr, j):
                # avT[dk, tokens] for head pair j, normalized by recip
                attn_uns = attn_tiles[pr]
                recip_rep = recip_tiles[pr]
                v_sb = v_tiles[pr]
                if j == 0:
                    avT_tiles[pr] = attnp.tile(
                        [P, HC, PTOK], BF16, tag="avT", name="avT")
                avT = avT_tiles[pr]
                avp = ps_proj.tile([P, PTOK], F32, tag="p512")
                for h in (2 * j, 2 * j + 1):
                    p0 = 64 * (h % 2)
                    sl = slice(p0, p0 + 64)
                    tpos = None if p0 == 0 else (0, 64)
                    first = True
                    for r in range(2):
                        for c in range(NCH):
                            nc.tensor.matmul(
                                avp[sl, r * S + STRIPE_START[c]:
                                        r * S + STRIPE_START[c] + STRIPE],
                                v_sb[:, 2 * r + c, h * DK:(h + 1) * DK],
                                attn_uns[h][:, c, r, :],
                                start=first,
                                stop=(r == 1 and c == NCH - 1),
                                tile_position=tpos,
                            )
                            first = False
                nc.vector.tensor_tensor(
                    avT[:, j, :], avp[:], recip_rep[:, j, :],
                    mybir.AluOpType.mult,
                )
                if j == HPAIRS - 1:
                    attn_tiles.pop(pr)
                    recip_tiles.pop(pr)
                    v_tiles.pop(pr)

            def stage_c(pr):
                avT = avT_tiles.pop(pr)
                o_sb = bpool.tile([P, TC, D], BF16, tag="o_sb")
                for tcc in range(TC):
                    pp = ps_proj.tile([P, PTOK], F32, tag="p512")
                    for hc in range(HC):
                        nc.tensor.matmul(
                            pp[:],
                            avT[:, hc, tcc * P:(tcc + 1) * P],
                            w2_sb[:, hc, :],
                            start=(hc == 0), stop=(hc == HC - 1),
                        )
                    if tcc % 2 == 0:
                        nc.scalar.activation(o_sb[:, tcc, :], pp[:], func=COPY)
                    else:
                        nc.vector.tensor_copy(o_sb[:, tcc, :], pp[:])
                    nc.sync.dma_start(
                        out_d[pr * PTOK + tcc * P:pr * PTOK + (tcc + 1) * P, :],
                        o_sb[:, tcc, :],
                    )

            # ---- software pipeline; scores/den/av interleaved per head so
            # PSUM-rotation waits are always covered by other PE work ----
            stage_load(0)
            load_w1qk()
            load_w1v()
            warmup_pe()
            if PAIRS > 1:
                stage_load(1)
            load_consts_rest()
            for t in range(PAIRS + 4):
                if t < PAIRS:
                    stage_a1(t)
                    if t + 2 < PAIRS:
                        stage_load(t + 2)
                for h in range(HEADS):
                    if 0 <= t - 1 < PAIRS:
                        scores_head(t - 1, h)
                        if t == 1 and h >= 1:
                            pe_filler(2)
                    if h < HPAIRS:
                        if 0 <= t - 2 < PAIRS:
                            den_group(t - 2, h)
                    else:
                        if 0 <= t - 3 < PAIRS:
                            av_group(t - 3, h - HPAIRS)
                if 0 <= t - 4 < PAIRS:
                    stage_c(t - 4)

    nc.compile()
    return nc


def _in_maps(inputs, pos_bias, W_qkv, W_out):
    x = np.asarray(inputs, np.float32).astype(ml_dtypes.bfloat16)
    W1qk, W1v, W2, expB, ecE = _host_constants(
        np.asarray(pos_bias), np.asarray(W_qkv), np.asarray(W_out))
    x_flat = x.reshape(B * HW, S, D)
    in_maps = []
    for core in range(N_CORES):
        shard = x_flat[core * ROWS_PER_CORE:(core + 1) * ROWS_PER_CORE]
        u = np.zeros((2, P), ml_dtypes.bfloat16)
        u[0, :64] = 1.0
        u[1, 64:] = 1.0
        in_maps.append({
            "x": np.ascontiguousarray(shard.reshape(ROWS_PER_CORE * S, D)),
            "w1qk": W1qk, "w1v": W1v, "w2": W2, "expb": expB, "ec": ecE,
            "u": u,
        })
    return in_maps


def kernel(inputs, pos_bias, W_qkv, W_out):
    if "nc" not in _CACHE:
        _CACHE["nc"] = _build_nc()
    nc = _CACHE["nc"]

    in_maps = _in_maps(inputs, pos_bias, W_qkv, W_out)
    res = run_bass_kernel_spmd(nc, in_maps, core_ids=list(range(N_CORES)))
    out = np.empty((B * HW, S, D), np.float32)
    for core in range(N_CORES):
        out[core * ROWS_PER_CORE:(core + 1) * ROWS_PER_CORE] = (
            res.results[core]["out"].astype(np.float32).reshape(
                ROWS_PER_CORE, S, D))
    return out.reshape(B, HW, S, D)


# revision 53
# speedup vs baseline: 1.0408x; 1.0408x over previous
"""Trainium2 Bass kernel for LocalSelfAttention (sliding-window attention).

Reference computation (fp32):
  qkv = x @ W_qkv ; q /= 8 ; sliding window of 7 keys (3 each side, zero-padded)
  attn = softmax(q . k_win + pos_bias) ; out = (attn @ v_win) @ W_out

Sharding: data-parallel over B*HW = 128 independent rows -> 16 rows per core.
Each core processes its rows in 8 pairs (512 tokens per pair).

Per-core layout, software-pipelined 5 pairs deep so the PE never stalls on
the vector-engine softmax chain:
  stage A1(p): xT arrives via DMA-transpose (x pre-cast to bf16 on host);
               qkT = W_qk^T. @ xT in fp8 DoubleRow (2 rows/cycle);
               V = xT^T. @ W_v (bf16)
  stage A2(p): scores ST[key,q] per head; exp w/ folded 1/sqrt(dk) and fp8
               descale (Scalar); * expB band mask (Pool)
  stage B1(p): denom = ones^T. @ attn_un (replicated across 64 partitions,
               2 heads per psum tile) + rank-2 matmul U^T. @ E adding the
               zero-pad edge correction; reciprocal_approx_fast from PSUM
  stage B2(p): avT = V^T. @ attn_un; * recip while copying PSUM->SBUF
  stage C(p):  out = avT^T. @ W_out -> DMA

Steady-state emission per step t: A1(t), then scores(t-1) interleaved
head-by-head with den(t-2)/av(t-3) groups (covers PSUM-rotation waits with
PE work), then out-proj C(t-4).
"""

import numpy as np
import ml_dtypes

import concourse.bass as bass
import concourse.tile as tile
from concourse import bacc, mybir
from concourse.bass_utils import run_bass_kernel_spmd

# Problem constants (hardcoded per contract)
B, HW, S, D = 2, 64, 256, 512
HEADS, DK, KSIZE, PAD = 8, 64, 7, 3
HDK = HEADS * DK            # 512
QK = 2 * HDK                # 1024 (q and k dims)
N_CORES = 8
ROWS_PER_CORE = (B * HW) // N_CORES   # 16
PAIRS = ROWS_PER_CORE // 2            # 8
PTOK = 2 * S                          # 512 tokens per pair
P = 128
NCH = S // P                          # 2 key chunks per row
STRIPE = 132                          # query stripe width per key chunk (even)
STRIPE_PAD = 256                      # psum slot per (chunk,row) stripe, bank aligned
STRIPE_START = (0, S - STRIPE)        # stripe start per chunk within a row
HPAIRS = HEADS // 2                   # 4 head pairs packed on 64+64 partitions

F32 = mybir.dt.float32
BF16 = mybir.dt.bfloat16
FP8 = mybir.dt.float8e4
FP8_WSCALE = 32.0                     # fp8 qk weight pre-scale (power of 2)
EXP_SCALE = 1.0 / (FP8_WSCALE * FP8_WSCALE * 8.0)  # undo w-scales + 1/sqrt(DK)

_CACHE = {}


def _host_constants(pos_bias, W_qkv, W_out):
    """Host-precomputed tensors: fp8 qk weights, bf16 v/out weights, expB
    band mask, rank-2 edge-correction factors."""
    W1 = W_qkv.astype(np.float32).copy()
    # qk weights scaled up by 32 to sit in fp8e4m3's normal range (std 0.02
    # would straddle the 2^-6 min normal); compensated in the exp scale,
    # which also folds in the 1/sqrt(DK) query scaling.
    W1qk = (W1[:, :QK] * FP8_WSCALE).astype(ml_dtypes.float8_e4m3)  # [512, 1024]
    W1v = W1[:, QK:].astype(ml_dtypes.bfloat16)               # [512, 512]
    W2 = W_out.astype(np.float32).astype(ml_dtypes.bfloat16)  # [512, 512]

    pb = pos_bias.astype(np.float32)              # [H, S, KSIZE]
    # expB[j, h, c, q'] : key j (within chunk c), query q = STRIPE_START[c] + q'
    # value exp(pos_bias[h, q, w]) with w = (j_global - q) + PAD if in band else 0
    j = np.arange(P)[:, None, None, None]
    h = np.arange(HEADS)[None, :, None, None]
    c = np.arange(NCH)[None, None, :, None]
    qp = np.arange(STRIPE)[None, None, None, :]
    q_glob = np.array(STRIPE_START)[None, None, :, None] + qp
    j_glob = c * P + j
    w = j_glob - q_glob + PAD
    in_band = (w >= 0) & (w < KSIZE)
    w_c = np.clip(w, 0, KSIZE - 1)
    bias_val = pb[h, q_glob, w_c]
    expB = np.where(in_band, np.exp(bias_val), 0.0).astype(np.float32)
    expB = expB.astype(ml_dtypes.bfloat16)        # [128, H, NCH, STRIPE]

    # edge correction: sum over out-of-range window slots of exp(bias).
    # Shipped as the moving operand E of a rank-2 matmul U^T. @ E that
    # accumulates it straight into the denominator PSUM: row i of E is the
    # correction for head 2j+i, row i of U selects partition half i.
    q = np.arange(S)[None, :, None]
    w2 = np.arange(KSIZE)[None, None, :]
    oor = ((q + w2 - PAD) < 0) | ((q + w2 - PAD) >= S)
    ec = (np.exp(pb) * oor).sum(-1)               # [H, S]
    ec_pair = np.concatenate([ec, ec], axis=1)    # [H, PTOK]
    ecE = np.empty((2, HPAIRS, PTOK), np.float32)
    for jj in range(HPAIRS):
        ecE[0, jj, :] = ec_pair[2 * jj]
        ecE[1, jj, :] = ec_pair[2 * jj + 1]
    return W1qk, W1v, W2, expB, ecE.astype(ml_dtypes.bfloat16)


def _build_nc():
    nc = bacc.Bacc(None, target_bir_lowering=False)
    x_d = nc.dram_tensor("x", [ROWS_PER_CORE * S, D], BF16, kind="ExternalInput")
    w1qk_d = nc.dram_tensor("w1qk", [D, QK], FP8, kind="ExternalInput")
    w1v_d = nc.dram_tensor("w1v", [D, HDK], BF16, kind="ExternalInput")
    w2_d = nc.dram_tensor("w2", [HDK, D], BF16, kind="ExternalInput")
    expb_d = nc.dram_tensor("expb", [P, HEADS, NCH, STRIPE], BF16, kind="ExternalInput")
    ec_d = nc.dram_tensor("ec", [2, HPAIRS, PTOK], BF16, kind="ExternalInput")
    u_d = nc.dram_tensor("u", [2, P], BF16, kind="ExternalInput")
    out_d = nc.dram_tensor("out", [ROWS_PER_CORE * S, D], BF16, kind="ExternalOutput")

    KO = D // P      # 4 K-chunks for projections
    TC = PTOK // P   # 4 token chunks per pair
    QKC = QK // P    # 8 qk output chunks
    HC = HDK // P    # 4 hdk chunks
    DR = mybir.MatmulPerfMode.DoubleRow
    COPY = mybir.ActivationFunctionType.Copy

    with tile.TileContext(nc) as tc:
        with (
            tc.tile_pool(name="const", bufs=1) as const,
            tc.tile_pool(name="io", bufs=3) as io,
            tc.tile_pool(name="early", bufs=2) as early,
            tc.tile_pool(name="vpool", bufs=3) as vpool,
            tc.tile_pool(name="attn", bufs=3) as attnp,
            tc.tile_pool(name="bpool", bufs=2) as bpool,
            tc.tile_pool(name="ps_proj", bufs=3, space="PSUM") as ps_proj,
            tc.tile_pool(name="ps_st", bufs=2, space="PSUM") as ps_st,
        ):
            # ---- constants; first x transpose goes ahead of the fat consts
            # (everything rides the sync queue, scalar stays free for copies)
            w1qk_sb = const.tile([P, KO, QK], FP8)
            w1v_sb = const.tile([P, KO, HDK], BF16)
            expb_sb = const.tile([P, HEADS, NCH, STRIPE], BF16)
            w2_sb = const.tile([P, HC, D], BF16)
            ecE_sb = const.tile([2, HPAIRS, PTOK], BF16)
            u_sb = const.tile([2, P], BF16)
            ones_sb = const.tile([P, 64], BF16)
            nc.vector.memset(ones_sb, 1.0)

            def load_w1qk():
                nc.sync.dma_start(
                    w1qk_sb[:], w1qk_d.rearrange("(ko ki) n -> ki ko n", ki=P))

            def load_w1v():
                nc.sync.dma_start(
                    w1v_sb[:], w1v_d.rearrange("(ko ki) n -> ki ko n", ki=P))

            warm = {}

            def warmup_pe():
                # dummy matmuls during the initial DMA wait: the PE needs ~3us
                # of continuous busy to reach its full 2.4GHz p-state
                scratch = const.tile([P, PTOK], BF16, name="warm_scr")
                nc.gpsimd.memset(scratch, 0.0)
                wps = ps_proj.tile([P, PTOK], F32, tag="warm", bufs=1, name="wps")
                warm["scratch"], warm["wps"] = scratch, wps
                for i in range(23):
                    nc.tensor.matmul(
                        wps[0:64, :], ones_sb[:], scratch[:],
                        start=True, stop=True,
                    )

            def pe_filler(n):
                # keep the PE busy (and its p-state hot) through pipeline-fill
                # bubbles where no other matmul work exists yet
                for i in range(n):
                    nc.tensor.matmul(
                        warm["wps"][0:64, :], ones_sb[:], warm["scratch"][:],
                        start=True, stop=True,
                    )

            def load_consts_rest():
                nc.sync.dma_start(expb_sb[:], expb_d[:])
                nc.sync.dma_start(
                    w2_sb[:], w2_d.rearrange("(hc ki) n -> ki hc n", ki=P))
                nc.sync.dma_start(ecE_sb[:], ec_d[:])
                nc.sync.dma_start(u_sb[:], u_d[:])

            xT_tiles = {}
            qkT_tiles = {}
            attn_tiles = {}
            recip_tiles = {}
            v_tiles = {}
            avT_tiles = {}

            def stage_load(pr, interleave=None):
                # DMA-transpose: x [tokens, D] bf16 -> xT[p, ko, t] = x[t, ko*128+p].
                # Two half-transposes so the fp8 cast can chase the transfer;
                # `interleave` slots a const DMA between them (in-order queue).
                xT = io.tile([P, KO, PTOK], BF16, tag="xT")
                for hf in range(2):
                    nc.sync.dma_start_transpose(
                        xT[:, :, hf * S:(hf + 1) * S],
                        x_d[pr * PTOK + hf * S:pr * PTOK + (hf + 1) * S, :])
                    if interleave:
                        interleave[hf]()
                xT_tiles[pr] = xT

            def stage_a1(pr):
                xT = xT_tiles.pop(pr)
                xT8 = early.tile([P, KO, PTOK], FP8, tag="xT8")
                for hf in range(2):
                    nc.vector.tensor_copy(
                        xT8[:, :, hf * S:(hf + 1) * S],
                        xT[:, :, hf * S:(hf + 1) * S])

                # qk projection in fp8 DoubleRow: qkT [qk dims, tokens]
                qkT = early.tile([P, QKC, PTOK], BF16, tag="qkT", bufs=3)
                qkT_tiles[pr] = qkT
                for m in range(QKC):
                    pp = ps_proj.tile([P, PTOK], F32, tag="p512")
                    for kp in range(KO // 2):
                        nc.tensor.matmul(
                            pp[:],
                            w1qk_sb[:, 2 * kp:2 * kp + 2, m * P:(m + 1) * P],
                            xT8[:, 2 * kp:2 * kp + 2, :],
                            start=(kp == 0), stop=(kp == KO // 2 - 1),
                            perf_mode=DR,
                        )
                    if m % 2 == 0:
                        nc.scalar.activation(qkT[:, m, :], pp[:], func=COPY)
                    else:
                        nc.vector.tensor_copy(qkT[:, m, :], pp[:])

                # v projection (bf16): V [tokens, hdk]
                v_sb = vpool.tile([P, TC, HDK], BF16, tag="v_sb", bufs=4)
                v_tiles[pr] = v_sb
                for tcc in range(TC):
                    pp = ps_proj.tile([P, PTOK], F32, tag="p512")
                    for ko in range(KO):
                        nc.tensor.matmul(
                            pp[:],
                            xT[:, ko, tcc * P:(tcc + 1) * P],
                            w1v_sb[:, ko, :],
                            start=(ko == 0), stop=(ko == KO - 1),
                        )
                    if tcc % 2 == 0:
                        nc.scalar.activation(v_sb[:, tcc, :], pp[:], func=COPY)
                    else:
                        nc.vector.tensor_copy(v_sb[:, tcc, :], pp[:])

            def scores_head(pr, h):
                # one head's scores + exp + band mask
                qkT = qkT_tiles[pr]
                mq = h // 2          # q chunk index in qkT
                mk = 4 + h // 2      # k chunk index in qkT
                p0 = 64 * (h % 2)    # partition offset within chunk
                sl = slice(p0, p0 + 64)

                st = ps_st.tile([P, NCH, 2, STRIPE_PAD], F32, tag="st")
                for c in range(NCH):
                    for r in range(2):
                        nc.tensor.matmul(
                            st[:, c, r, :STRIPE],
                            qkT[sl, mk, r * S + c * P:r * S + (c + 1) * P],
                            qkT[sl, mq,
                                r * S + STRIPE_START[c]:
                                r * S + STRIPE_START[c] + STRIPE],
                            start=True, stop=True,
                        )
                attn_un = attnp.tile([P, NCH, 2, STRIPE], BF16, tag=f"attn_un{h}")
                nc.scalar.activation(
                    attn_un[:], st[:, :, :, :STRIPE],
                    func=mybir.ActivationFunctionType.Exp,
                    scale=EXP_SCALE)
                nc.gpsimd.tensor_tensor(
                    attn_un[:], attn_un[:],
                    expb_sb[:, h, :, None, :].to_broadcast((P, NCH, 2, STRIPE)),
                    mybir.AluOpType.mult,
                )
                attn_tiles.setdefault(pr, []).append(attn_un)

            def den_group(pr, j):
                # denominators for head pair j: ones-matmuls + rank-2 edge
                # correction accumulated in PSUM, then approx reciprocal
                attn_uns = attn_tiles[pr]
                if j == 0:
                    recip_tiles[pr] = bpool.tile(
                        [P, HPAIRS, PTOK], F32, tag="recip_rep",
                        name="recip_rep")
                recip_rep = recip_tiles[pr]
                den = ps_proj.tile([P, PTOK], F32, tag="p512")
                for h in (2 * j, 2 * j + 1):
                    p0 = 64 * (h % 2)
                    sl = slice(p0, p0 + 64)
                    tpos = None if p0 == 0 else (0, 64)
                    for r in range(2):
                        for c in range(NCH):
                            nc.tensor.matmul(
                                den[sl, r * S + STRIPE_START[c]:
                                        r * S + STRIPE_START[c] + STRIPE],
                                ones_sb[:],
                                attn_uns[h][:, c, r, :],
                                start=(r == 0 and c == 0),
                                stop=False,
                                tile_position=tpos,
                            )
                nc.tensor.matmul(
                    den[:], u_sb[:, :], ecE_sb[:, j, :],
                    start=False, stop=True,
                )
                nc.vector.reciprocal_approx_fast(recip_rep[:, j, :], den[:])

            def av_group(pr, j):
                # avT[dk, tokens] for head pair j, normalized by recip
                attn_uns = attn_tiles[pr]
                recip_rep = recip_tiles[pr]
                v_sb = v_tiles[pr]
                if j == 0:
                    avT_tiles[pr] = attnp.tile(
                        [P, HC, PTOK], BF16, tag="avT", name="avT")
                avT = avT_tiles[pr]
                avp = ps_proj.tile([P, PTOK], F32, tag="p512")
                for h in (2 * j, 2 * j + 1):
                    p0 = 64 * (h % 2)
                    sl = slice(p0, p0 + 64)
                    tpos = None if p0 == 0 else (0, 64)
                    first = True
                    for r in range(2):
                        for c in range(NCH):
                            nc.tensor.matmul(
                                avp[sl, r * S + STRIPE_START[c]:
                                        r * S + STRIPE_START[c] + STRIPE],
                                v_sb[:, 2 * r + c, h * DK:(h + 1) * DK],
                                attn_uns[h][:, c, r, :],
                                start=first,
                                stop=(r == 1 and c == NCH - 1),
                                tile_position=tpos,
                            )
                            first = False
                nc.vector.tensor_tensor(
                    avT[:, j, :], avp[:], recip_rep[:, j, :],
                    mybir.AluOpType.mult,
                )
                if j == HPAIRS - 1:
                    attn_tiles.pop(pr)
                    recip_tiles.pop(pr)
                    v_tiles.pop(pr)

            def stage_c(pr):
                avT = avT_tiles.pop(pr)
                o_sb = bpool.tile([P, TC, D], BF16, tag="o_sb")
                for tcc in range(TC):
                    pp = ps_proj.tile([P, PTOK], F32, tag="p512")
                    for hc in range(HC):
                        nc.tensor.matmul(
                            pp[:],
                            avT[:, hc, tcc * P:(tcc + 1) * P],
                            w2_sb[:, hc, :],
                            start=(hc == 0), stop=(hc == HC - 1),
                        )
                    if tcc % 2 == 0:
                        nc.scalar.activation(o_sb[:, tcc, :], pp[:], func=COPY)
                    else:
                        nc.vector.tensor_copy(o_sb[:, tcc, :], pp[:])
                    nc.sync.dma_start(
                        out_d[pr * PTOK + tcc * P:pr * PTOK + (tcc + 1) * P, :],
                        o_sb[:, tcc, :],
                    )

            # ---- software pipeline; scores/den/av interleaved per head so
            # PSUM-rotation waits are always covered by other PE work ----
            stage_load(0)
            load_w1qk()
            load_w1v()
            warmup_pe()
            if PAIRS > 1:
                stage_load(1)
            load_consts_rest()
            for t in range(PAIRS + 4):
                if t < PAIRS:
                    stage_a1(t)
                    if t + 2 < PAIRS:
                        stage_load(t + 2)
                for h in range(HEADS):
                    if 0 <= t - 1 < PAIRS:
                        scores_head(t - 1, h)
                        if t == 1 and h >= 1:
                            pe_filler(2)
                    if h < HPAIRS:
                        if 0 <= t - 2 < PAIRS:
                            den_group(t - 2, h)
                    else:
                        if 0 <= t - 3 < PAIRS:
                            av_group(t - 3, h - HPAIRS)
                if 0 <= t - 4 < PAIRS:
                    stage_c(t - 4)

    nc.compile()
    return nc


def _in_maps(inputs, pos_bias, W_qkv, W_out):
    x = np.asarray(inputs, np.float32).astype(ml_dtypes.bfloat16)
    W1qk, W1v, W2, expB, ecE = _host_constants(
        np.asarray(pos_bias), np.asarray(W_qkv), np.asarray(W_out))
    x_flat = x.reshape(B * HW, S, D)
    in_maps = []
    for core in range(N_CORES):
        shard = x_flat[core * ROWS_PER_CORE:(core + 1) * ROWS_PER_CORE]
        u = np.zeros((2, P), ml_dtypes.bfloat16)
        u[0, :64] = 1.0
        u[1, 64:] = 1.0
        in_maps.append({
            "x": np.ascontiguousarray(shard.reshape(ROWS_PER_CORE * S, D)),
            "w1qk": W1qk, "w1v": W1v, "w2": W2, "expb": expB, "ec": ecE,
            "u": u,
        })
    return in_maps


def kernel(inputs, pos_bias, W_qkv, W_out):
    if "nc" not in _CACHE:
        _CACHE["nc"] = _build_nc()
    nc = _CACHE["nc"]

    in_maps = _in_maps(inputs, pos_bias, W_qkv, W_out)
    res = run_bass_kernel_spmd(nc, in_maps, core_ids=list(range(N_CORES)))
    out = np.empty((B * HW, S, D), np.float32)
    for core in range(N_CORES):
        out[core * ROWS_PER_CORE:(core + 1) * ROWS_PER_CORE] = (
            res.results[core]["out"].astype(np.float32).reshape(
                ROWS_PER_CORE, S, D))
    return out.reshape(B, HW, S, D)
